# revision 8
# baseline (speedup 1.0000x reference)
"""MultiHeadAttention Trainium2 Bass kernel (v2.2).

Model: B=2, S=2048, D_MODEL=1024, H=16 heads, Dh=64.
  q/k/v = x @ W.T + b ; scores = (q k^T)/8 masked-softmax ; out = w @ v ; y = out @ Wy.T + by

Sharding: (batch x sequence) data parallel over 8 cores. Core c handles
batch b = c // 4 and query rows [q0, q0+512) with q0 = (c % 4) * 512.
K/V projections are computed (replicated) per batch on each core; attention
and the output projection are computed only for the core's query slice, so
the final output is a pure concatenation of per-core slices.

Design notes:
  - Mixed precision: the query path (qT, Wq, Q-proj, QT) and the projected
    K tiles are f32r (score errors on the q side act like a perturbed query
    and do NOT average out over keys in softmax; k-side input errors do).
    K/V/Y inputs+weights and the attention-weight/value path are bf16.
    PSUM accumulation is always f32.
  - V stays in SBUF (no DRAM round trip): Vfull[p_tok, st, h, 65] with a
    built-in ones column per head so the AV matmul also produces softmax
    denominators in PSUM partition 64 for free (cost = moving columns).
  - Mask applied multiplicatively AFTER exp: Act does exp(PSUM)->bf16 SBUF,
    DVE multiplies by a bf16 {0,1} mask (2x DVE mode on 2-byte SBUF
    operands). Masked weights are exactly 0, matching the reference.
  - Softmax division: DVE reciprocal of the denominator row, PE broadcasts
    it across partitions with a tiny fp32 ones-stationary matmul, Act
    evacuates the broadcast to SBUF (PSUM has one DVE read port), one DVE
    multiply writes attnT.
  - All projection biases on DVE so Act runs exp-only during attention
    (no activation-table swaps).
  - K-projection is m-sliced (128 output dims = 2 heads) and interleaved
    round-by-round with attention for those heads so PE/Act/DVE/DMA overlap
    across the kernel instead of running in serial phases.
"""

import numpy as np
from ml_dtypes import bfloat16

import concourse.bass as bass
import concourse.mybir as mybir
import concourse.tile as tile
from concourse import bacc
from concourse.bass_utils import run_bass_kernel_spmd

F32 = mybir.dt.float32
F32R = mybir.dt.float32r
BF16 = mybir.dt.bfloat16

B, S, D, H, DH = 2, 2048, 1024, 16, 64
QS = 512          # query rows per core
P = 128
KO = D // P       # 8 contraction tiles for the projections
NKT = S // P      # 16 key tiles

_CACHE = {}


def build_program():
    nc = bacc.Bacc("TRN2", target_bir_lowering=False, debug=False, num_devices=8)

    # ---- external I/O (per-core shapes) ----
    qT = nc.dram_tensor("qT", [D, QS], F32R, kind="ExternalInput")       # queries[b].T slice
    kT = nc.dram_tensor("kT", [D, S], BF16, kind="ExternalInput")        # keys[b].T
    vT = nc.dram_tensor("vT", [D, S], BF16, kind="ExternalInput")        # values[b].T
    maskb = nc.dram_tensor("maskb", [H, S, QS], BF16, kind="ExternalInput")  # {0,1}, [h,k,q]
    WqT = nc.dram_tensor("WqT", [D, D], F32R, kind="ExternalInput")      # (Wq/8).T
    WkT = nc.dram_tensor("WkT", [D, D], BF16, kind="ExternalInput")
    WvT = nc.dram_tensor("WvT", [D, D], BF16, kind="ExternalInput")
    WyT = nc.dram_tensor("WyT", [D, D], BF16, kind="ExternalInput")
    bq = nc.dram_tensor("bq", [P, KO], F32, kind="ExternalInput")        # ((bq+bq2)/8) as [p, m]
    bk = nc.dram_tensor("bk", [P, KO], F32, kind="ExternalInput")
    bv = nc.dram_tensor("bv", [1, D], BF16, kind="ExternalInput")
    by = nc.dram_tensor("by", [1, D], BF16, kind="ExternalInput")
    y = nc.dram_tensor("y", [QS, D], F32, kind="ExternalOutput")

    qT_r = qT.rearrange("(ko p) q -> p ko q", p=P)
    kT_r = kT.rearrange("(ko p) s -> p ko s", p=P)
    vT_r = vT.rearrange("(ko p) s -> p ko s", p=P)
    maskb_r = maskb.rearrange("h (kt p) q -> h p kt q", p=P)
    WqT_r = WqT.rearrange("(ko p) m -> p ko m", p=P)
    WkT_r = WkT.rearrange("(ko p) m -> p ko m", p=P)
    WvT_r = WvT.rearrange("(ko p) m -> p ko m", p=P)
    WyT_r = WyT.rearrange("(ko p) m -> p ko m", p=P)

    def bcast_dram(ap, parts):
        # partition-broadcast AP: read the same DRAM row into `parts` partitions
        return bass.AP(tensor=ap.tensor, offset=ap.offset, ap=[[0, parts]] + list(ap.ap[1:]))

    with tile.TileContext(nc) as tc:
        with (
            tc.tile_pool(name="persist", bufs=1) as persist,
            tc.tile_pool(name="w", bufs=2) as wpool,
            tc.tile_pool(name="qcol", bufs=2) as qcolp,
            tc.tile_pool(name="ktm", bufs=2) as ktp,
            tc.tile_pool(name="maskp", bufs=4) as maskp,
            tc.tile_pool(name="eT", bufs=3) as eTp,
            tc.tile_pool(name="rec", bufs=2) as recp,
            tc.tile_pool(name="outp", bufs=2) as outp,
            tc.tile_pool(name="psP", bufs=2, space="PSUM") as psP,
            tc.tile_pool(name="psS", bufs=3, space="PSUM") as psS,
            tc.tile_pool(name="psT", bufs=2, space="PSUM") as psT,
        ):
            # ---- persistent SBUF ----
            kTsb = persist.tile([P, KO, S], BF16)        # 32 KB/part: keys^T staged
            Vfull = persist.tile([P, NKT, H, DH + 1], BF16)  # 32.5 KB/part: V + ones col
            QT = persist.tile([P, KO, QS], F32R)         # 16 KB/part
            attnT = persist.tile([P, KO, QS], BF16)      # 8 KB/part
            bq_sb = persist.tile([P, KO], F32)
            bk_sb = persist.tile([P, KO], F32)
            bv_bc = persist.tile([P, D], BF16)
            by_bc = persist.tile([P, D], BF16)
            ones64 = persist.tile([1, DH], F32)

            nc.sync.dma_start(out=bq_sb, in_=bq[:])
            nc.sync.dma_start(out=bk_sb, in_=bk[:])
            nc.sync.dma_start(out=bv_bc, in_=bcast_dram(bv[:], P))
            nc.sync.dma_start(out=by_bc, in_=bcast_dram(by[:], P))
            nc.vector.memset(ones64[:], 1.0)
            nc.vector.memset(Vfull[:, :, :, DH:DH + 1], 1.0)

            # queries^T staged in two f32r chunks (pinned through phase Q)
            qcols = []
            for nchq in range(2):
                qc = qcolp.tile([P, KO, 256], F32R, tag="qcol")
                nc.sync.dma_start(out=qc, in_=qT_r[:, :, nchq * 256:(nchq + 1) * 256])
                qcols.append(qc)
            # keys^T staged early (needed from round 0 on), in 4 chunks
            for c in range(4):
                nc.sync.dma_start(out=kTsb[:, :, c * 512:(c + 1) * 512],
                                  in_=kT_r[:, :, c * 512:(c + 1) * 512])
            wv = wpool.tile([P, KO, D], BF16, tag="w")
            nc.sync.dma_start(out=wv, in_=WvT_r[:])

            # ---- phase Q: QT[dout_p, m, q] = (Wq/8) @ queries[b].T + bq/8 ----
            for m in range(KO):
                wqm = ktp.tile([P, KO, P], F32R, tag="ktm")
                nc.sync.dma_start(out=wqm, in_=WqT_r[:, :, m * P:(m + 1) * P])
                ps = psP.tile([P, QS], F32, tag="proj")
                for nchq in range(2):
                    for ko in range(KO):
                        nc.tensor.matmul(
                            ps[:, nchq * 256:(nchq + 1) * 256], wqm[:, ko, :],
                            qcols[nchq][:, ko, :],
                            start=(ko == 0), stop=(ko == KO - 1))
                nc.vector.tensor_scalar(
                    QT[:, m, :], ps[:], bq_sb[:, m:m + 1], None, mybir.AluOpType.add)

            # wk loads during the V phase (reuses wv's other slot)
            wk = wpool.tile([P, KO, D], BF16, tag="w")
            nc.sync.dma_start(out=wk, in_=WkT_r[:])
            # round-0 masks prefetch during V phase
            mtiles = {}
            for hp in range(2):
                for half in range(2):
                    mt = maskp.tile([P, KO, QS], BF16, tag="mask")
                    nc.sync.dma_start(out=mt, in_=maskb_r[hp, :, half * 8:(half + 1) * 8, :])
                    mtiles[(0, hp, half)] = mt

            # ---- phase V: Vfull[p_tok, st, h, d] = values[b] @ Wv.T + bv ----
            for stc in range(8):   # two key tiles per streamed chunk
                vch = ktp.tile([P, KO, 2 * P], BF16, tag="ktm")
                nc.sync.dma_start(out=vch, in_=vT_r[:, :, stc * 256:(stc + 1) * 256])
                for st2 in range(2):
                    st = stc * 2 + st2
                    for half in range(2):
                        ps = psP.tile([P, QS], F32, tag="proj")
                        for ko in range(KO):
                            nc.tensor.matmul(
                                ps[:], vch[:, ko, st2 * P:(st2 + 1) * P],
                                wv[:, ko, half * 512:(half + 1) * 512],
                                start=(ko == 0), stop=(ko == KO - 1))
                        nc.vector.tensor_tensor(
                            Vfull[:, st, half * 8:(half + 1) * 8, 0:DH],
                            ps.rearrange("p (h d) -> p h d", d=DH),
                            bv_bc[:, half * 512:(half + 1) * 512].rearrange(
                                "p (h d) -> p h d", d=DH),
                            mybir.AluOpType.add)

            # ---- rounds: K-proj for 2 heads + attention for those heads ----
            for m in range(KO):
                # K_m: ktm[dout_p(128 = heads 2m,2m+1), k] = Wk_m @ keys^T + bk_m
                ktm = ktp.tile([P, S], F32R, tag="ktm")
                for nch2 in range(4):
                    ps = psP.tile([P, QS], F32, tag="proj")
                    for ko in range(KO):
                        nc.tensor.matmul(
                            ps[:], wk[:, ko, m * P:(m + 1) * P],
                            kTsb[:, ko, nch2 * 512:(nch2 + 1) * 512],
                            start=(ko == 0), stop=(ko == KO - 1))
                    nc.vector.tensor_scalar(
                        ktm[:, nch2 * 512:(nch2 + 1) * 512], ps[:],
                        bk_sb[:, m:m + 1], None, mybir.AluOpType.add)

                # prefetch next round's masks
                if m + 1 < KO:
                    for hp in range(2):
                        for half in range(2):
                            mt = maskp.tile([P, KO, QS], BF16, tag="mask")
                            nc.sync.dma_start(
                                out=mt,
                                in_=maskb_r[2 * (m + 1) + hp, :, half * 8:(half + 1) * 8, :])
                            mtiles[(m + 1, hp, half)] = mt

                for hp in range(2):
                    h = 2 * m + hp
                    m0 = mtiles[(m, hp, 0)]
                    m1 = mtiles[(m, hp, 1)]
                    qh = QT[hp * DH:(hp + 1) * DH, m, :]
                    patt = psT.tile([P, QS], F32, tag="att")
                    for kt in range(NKT):
                        pscr = psS.tile([P, QS], F32, tag="sc")
                        nc.tensor.matmul(
                            pscr[:], ktm[hp * DH:(hp + 1) * DH, kt * P:(kt + 1) * P],
                            qh, start=True, stop=True)
                        eT = eTp.tile([P, QS], BF16, tag="eT")
                        nc.scalar.activation(out=eT[:], in_=pscr[:],
                                             func=mybir.ActivationFunctionType.Exp)
                        mt = m0 if kt < 8 else m1
                        eTm = eTp.tile([P, QS], BF16, tag="eT")
                        nc.vector.tensor_tensor(eTm[:], eT[:], mt[:, kt % 8, :],
                                                mybir.AluOpType.mult)
                        nc.tensor.matmul(
                            patt[0:DH + 1, :], Vfull[:, kt, h, :], eTm[:],
                            start=(kt == 0), stop=(kt == NKT - 1))
                    # softmax denominator: reciprocal + partition-broadcast via PE
                    rec = recp.tile([1, QS], F32, tag="rec")
                    nc.vector.reciprocal(out=rec[:], in_=patt[DH:DH + 1, :])
                    pbc = psS.tile([P, QS], F32, tag="sc")
                    nc.tensor.matmul(
                        pbc[0:DH, :], ones64[:], rec[:], start=True, stop=True)
                    # PSUM has one DVE read port: evacuate the broadcast to SBUF
                    # (Act) so the final multiply reads a single PSUM operand.
                    rb = recp.tile([DH, QS], F32, tag="rb")
                    nc.scalar.activation(out=rb[:], in_=pbc[0:DH, :],
                                         func=mybir.ActivationFunctionType.Copy)
                    nc.vector.tensor_tensor(
                        attnT[hp * DH:(hp + 1) * DH, m, :], patt[0:DH, :], rb[:],
                        mybir.AluOpType.mult)

            # ---- phase Y: y = merged @ Wy.T + by ----
            for nch in range(4):
                wy = ktp.tile([P, KO, 256], BF16, tag="ktm")
                nc.sync.dma_start(out=wy, in_=WyT_r[:, :, nch * 256:(nch + 1) * 256])
                for mq in range(4):
                    ps = psP.tile([P, QS], F32, tag="proj")
                    for ko in range(KO):
                        nc.tensor.matmul(
                            ps[:, 0:256], attnT[:, ko, mq * P:(mq + 1) * P],
                            wy[:, ko, :], start=(ko == 0), stop=(ko == KO - 1))
                    ysb = outp.tile([P, 256], F32, tag="ysb")
                    nc.vector.tensor_tensor(
                        ysb[:], ps[:, 0:256], by_bc[:, nch * 256:(nch + 1) * 256],
                        mybir.AluOpType.add)
                    nc.sync.dma_start(
                        out=y[mq * P:(mq + 1) * P, nch * 256:(nch + 1) * 256], in_=ysb[:])

    nc.compile()
    return nc


def prep_inputs(queries, keys, values, mask, Wq, bq, Wk, bk, Wv, bv, Wy, by,
                bq2, bk2, bv2, by2):
    f = np.float32
    WqT = np.ascontiguousarray((Wq.astype(f) / 8.0).T)
    WkT = np.ascontiguousarray(Wk.astype(f).T).astype(bfloat16)
    WvT = np.ascontiguousarray(Wv.astype(f).T).astype(bfloat16)
    WyT = np.ascontiguousarray(Wy.astype(f).T).astype(bfloat16)
    bq_t = np.ascontiguousarray(((bq + bq2).astype(f) / 8.0).reshape(KO, P).T)
    bk_t = np.ascontiguousarray((bk + bk2).astype(f).reshape(KO, P).T)
    bv_t = np.ascontiguousarray((bv + bv2).astype(f)[None, :]).astype(bfloat16)
    by_t = np.ascontiguousarray((by + by2).astype(f)[None, :]).astype(bfloat16)

    qT = [np.ascontiguousarray(queries[b].astype(f).T) for b in range(B)]
    kT = [np.ascontiguousarray(keys[b].astype(f).T).astype(bfloat16) for b in range(B)]
    vT = [np.ascontiguousarray(values[b].astype(f).T).astype(bfloat16) for b in range(B)]
    mbf = mask.astype(bfloat16)  # {0,1}

    in_maps = []
    for c in range(8):
        b, qi = c // 4, c % 4
        q0 = qi * QS
        in_maps.append({
            "qT": np.ascontiguousarray(qT[b][:, q0:q0 + QS]),
            "kT": kT[b],
            "vT": vT[b],
            "maskb": np.ascontiguousarray(mbf[b, :, q0:q0 + QS, :].transpose(0, 2, 1)),
            "WqT": WqT, "WkT": WkT, "WvT": WvT, "WyT": WyT,
            "bq": bq_t, "bk": bk_t, "bv": bv_t, "by": by_t,
        })
    return in_maps


def kernel(**inputs):
    if "nc" not in _CACHE:
        _CACHE["nc"] = build_program()
    nc = _CACHE["nc"]
    in_maps = prep_inputs(**inputs)
    res = run_bass_kernel_spmd(nc, in_maps, core_ids=list(range(8)))
    out = np.empty((B, S, D), dtype=np.float32)
    for c in range(8):
        b, qi = c // 4, c % 4
        out[b, qi * QS:(qi + 1) * QS, :] = res.results[c]["y"]
    return out
